# revision 1
# baseline (speedup 1.0000x reference)
"""Trainium2 Bass kernel for a 2-layer GAT (PyG GATConv semantics) + sigmoid head.

Strategy (8 NeuronCores, SPMD, single NEFF launch):
  - Nodes are block-sharded: core k owns a contiguous range of `wpc` windows of
    128 nodes.  Edges (with self-loops appended) are sorted by destination on
    the host and bucketed per (core, window); the segment softmax and the
    message aggregation are therefore fully core-local.
  - Dense projections are node-sharded.  Host-side weight augmentation
    [W | W@A_src_blk | W@A_dst_blk (| W@Wp_blk)] makes every per-node scalar
    (attention logits contributions, final-projection dot) a column of one
    matmul, so no on-chip transposes are needed to compute them.
  - Per-node tables (h rows + aux rows) are AllGathered across the 8 cores.
  - The edge stage gathers h[src] / aux[src] rows with bulk `dma_gather`
    (int16 indices, low/high table split to cover >32K rows), gathers
    aux[dst] from the core-local table slice, and does the segment
    sum (softmax numerator AND denominator) on the TensorEngine via
    selection-matrix matmuls accumulated in PSUM.
  - exp() without max-subtraction: attention logits here are O(1), far from
    fp32 overflow, and softmax is shift-invariant, so the segment-max pass
    of the reference is unnecessary.
  - Layer 2 never materializes h2: folding Wp into the aux table (p = h2@Wp
    per head) means layer-2 messages are 8 scalars/edge instead of 256.

kernel(**inputs) takes the FULL inputs and returns the FULL [N, 1] output.
"""

import math

import numpy as np

import concourse.bacc as bacc
import concourse.mybir as mybir
import concourse.tile as tile
from concourse import bass_utils
from concourse.masks import make_identity

F32 = mybir.dt.float32
F32R = mybir.dt.float32r
BF16 = mybir.dt.bfloat16
I16 = mybir.dt.int16
BF16_H = True   # h-table + message path in bf16 (halves gather traffic,
                # full-rate TensorE) — set False for an all-fp32 fallback

N_CORES = 8
WIN = 128           # nodes per window (= PSUM partition dim)
LOWCAP = 32768      # int16 index ceiling for dma_gather
EPS = 1e-16
NEG_SLOPE = 0.2

_CACHE: dict = {}

last_exec_seconds = None   # wall time of the device execution of the last call


def _pack_idx(vals: np.ndarray, nidx: int) -> np.ndarray:
    """int16 index grid for dma_gather: index j at [16g + j%16, j//16]."""
    assert vals.shape[0] == nidx and nidx % 16 == 0
    a = vals.astype(np.int16).reshape(nidx // 16, 16).T  # [16, nidx/16]
    return np.tile(a, (8, 1))                            # [128, nidx/16]


def _blk(a: np.ndarray, H: int, C: int) -> np.ndarray:
    """[H, C] head params -> [H*C, H] block-diagonal matrix."""
    out = np.zeros((H * C, H), np.float32)
    for h in range(H):
        out[h * C:(h + 1) * C, h] = a[h]
    return out


def _prep_host(edge_index, x, W1, as1, ad1, b1, W2, as2, ad2, b2, Wp, bp):
    N, F = x.shape
    H, C = as1.shape
    HC = H * C
    wpc = math.ceil(N / (N_CORES * WIN))
    NPC = wpc * WIN
    NPAD = N_CORES * NPC
    split = NPAD > LOWCAP

    # ---- augmented weights ----
    # layer 1: [W1 | W1@As | W1@Ad] with an extra all-ones input row carrying
    # [b1 | 0 | 0]  (softmax rows sum to 1, so adding b1 to every table row
    # reproduces `out + b1`).
    W1h = np.concatenate(
        [W1.astype(np.float32),
         W1.astype(np.float32) @ _blk(as1, H, C),
         W1.astype(np.float32) @ _blk(ad1, H, C)], axis=1)            # [F, HC+2H]
    D1 = HC + 2 * H
    K1 = F + 1
    K1p = math.ceil(K1 / 128) * 128
    W1h_pad = np.zeros((K1p, D1), np.float32)
    W1h_pad[:F] = W1h
    W1h_pad[F, :HC] = b1

    # layer 2: only [W2@As | W2@Ad | W2@Wp_blk]; h2 itself is never needed.
    W2f = W2.astype(np.float32)
    W2h = np.concatenate(
        [W2f @ _blk(as2, H, C), W2f @ _blk(ad2, H, C),
         W2f @ _blk(np.tile(Wp[:, 0][None, :], (H, 1)), H, C)], axis=1)  # [HC, 3H]
    D2 = 3 * H
    const2 = float(np.asarray(b2, np.float64) @ np.asarray(Wp[:, 0], np.float64)
                   + np.asarray(bp, np.float64)[0])

    # ---- edges ----
    loops = np.arange(N, dtype=np.int64)
    src = np.concatenate([np.asarray(edge_index[0]), loops])
    dst = np.concatenate([np.asarray(edge_index[1]), loops])
    order = np.argsort(dst, kind="stable")
    src_s = src[order]
    dst_s = dst[order]
    G = N_CORES * wpc
    wb = np.searchsorted(dst_s, np.arange(0, G * WIN + 1, WIN))

    lo_l, hi_l, d_l = [], [], []
    for g in range(G):
        s = src_s[wb[g]:wb[g + 1]]
        d = (dst_s[wb[g]:wb[g + 1]] - g * WIN).astype(np.int64)
        if split:
            m = s < LOWCAP
            lo_l.append((s[m], d[m]))
            hi_l.append((s[~m] - LOWCAP, d[~m]))
        else:
            lo_l.append((s, d))
            hi_l.append((s[:0], d[:0]))
        d_l.append(None)

    T_LO = max(1, max(math.ceil(len(a[0]) / WIN) for a in lo_l))
    T_HI = max(math.ceil(len(a[0]) / WIN) for a in hi_l) if split else 0
    T = T_LO + T_HI

    idxlo = np.zeros((N_CORES, 128, wpc * T_LO * 8), np.int16)
    idxhi = np.zeros((N_CORES, 128, max(1, wpc * T_HI * 8)), np.int16)
    idxd = np.zeros((N_CORES, 128, wpc * T * 8), np.int16)
    dstl = np.full((N_CORES, 128, wpc * T), 200.0, np.float32)

    for k in range(N_CORES):
        for w in range(wpc):
            g = k * wpc + w
            (slo, dlo), (shi, dhi) = lo_l[g], hi_l[g]
            nlo, nhi = len(slo), len(shi)
            flo = np.zeros(T_LO * WIN, np.int64)
            flo[:nlo] = slo
            idxlo[k, :, w * T_LO * 8:(w + 1) * T_LO * 8] = _pack_idx(flo, T_LO * WIN)
            if T_HI:
                fhi = np.zeros(T_HI * WIN, np.int64)
                fhi[:nhi] = shi
                idxhi[k, :, w * T_HI * 8:(w + 1) * T_HI * 8] = _pack_idx(fhi, T_HI * WIN)
            # full window order: [lo | lo-pad | hi | hi-pad]
            dall = np.full(T * WIN, 200.0, np.float32)
            dall[:nlo] = dlo
            dall[T_LO * WIN:T_LO * WIN + nhi] = dhi
            dstl[k, :, w * T:(w + 1) * T] = dall.reshape(T, WIN).T
            # dst-local indices (into this core's own table slice)
            dloc = np.zeros(T * WIN, np.int64)
            dloc[:nlo] = dlo + w * WIN
            dloc[T_LO * WIN:T_LO * WIN + nhi] = dhi + w * WIN
            idxd[k, :, w * T * 8:(w + 1) * T * 8] = _pack_idx(dloc, T * WIN)

    # ---- per-core dense inputs ----
    xT = np.zeros((N_CORES, K1p, NPC), np.float32)
    xnp = np.asarray(x, np.float32)
    for k in range(N_CORES):
        lo = k * NPC
        hi = min(N, (k + 1) * NPC)
        if hi > lo:
            xT[k, :F, :hi - lo] = xnp[lo:hi].T
        xT[k, F, :] = 1.0

    iota = np.tile(np.arange(128, dtype=np.float32)[None, :], (128, 1))

    meta = dict(N=N, F=F, H=H, C=C, HC=HC, wpc=wpc, NPC=NPC, NPAD=NPAD,
                split=split, T_LO=T_LO, T_HI=T_HI, T=T, K1p=K1p, D1=D1, D2=D2,
                const2=const2)
    in_maps = []
    for k in range(N_CORES):
        in_maps.append({
            "xT": xT[k], "idxlo": idxlo[k], "idxhi": idxhi[k],
            "idxd": idxd[k], "dstl": dstl[k],
            "W1h": W1h_pad, "W2h": W2h.astype(np.float32), "iota": iota,
        })
    return meta, in_maps


def _build(meta, reps=1):
    import os
    SKIP_E2 = bool(int(os.environ.get("GAT_SKIP_E2", "0")))
    SKIP_E1C = bool(int(os.environ.get("GAT_SKIP_E1C", "0")))  # gathers only in e1
    SKIP_E1 = bool(int(os.environ.get("GAT_SKIP_E1", "0")))
    SKIP_DENSE = bool(int(os.environ.get("GAT_SKIP_DENSE", "0")))
    """Build + compile the SPMD Bass module for the given meta config.

    reps > 1 unrolls the whole computation body multiple times inside one
    NEFF; used to measure device time as a slope, cancelling the ~100ms
    PJRT/axon dispatch overhead.
    """
    H, C, HC = meta["H"], meta["C"], meta["HC"]
    wpc, NPC, NPAD = meta["wpc"], meta["NPC"], meta["NPAD"]
    T_LO, T_HI, T = meta["T_LO"], meta["T_HI"], meta["T"]
    K1p, D1, D2 = meta["K1p"], meta["D1"], meta["D2"]
    split, const2 = meta["split"], meta["const2"]
    KC1 = K1p // 128
    KC2 = HC // 128

    nc = bacc.Bacc("TRN2", target_bir_lowering=False, debug=False,
                   num_devices=N_CORES)
    xT = nc.dram_tensor("xT", [K1p, NPC], F32, kind="ExternalInput").ap()
    idxlo = nc.dram_tensor("idxlo", [128, wpc * T_LO * 8], I16,
                           kind="ExternalInput").ap()
    idxhi = nc.dram_tensor("idxhi", [128, max(1, wpc * T_HI * 8)], I16,
                           kind="ExternalInput").ap()
    idxd = nc.dram_tensor("idxd", [128, wpc * T * 8], I16,
                          kind="ExternalInput").ap()
    dstl = nc.dram_tensor("dstl", [128, wpc * T], F32, kind="ExternalInput").ap()
    W1h = nc.dram_tensor("W1h", [K1p, D1], F32, kind="ExternalInput").ap()
    W2h = nc.dram_tensor("W2h", [HC, D2], F32, kind="ExternalInput").ap()
    iota_in = nc.dram_tensor("iota", [128, 128], F32, kind="ExternalInput").ap()
    zout = nc.dram_tensor("zout", [NPC, 1], F32, kind="ExternalOutput").ap()

    AF = mybir.ActivationFunctionType
    OP = mybir.AluOpType

    with tile.TileContext(nc) as tc:
        with (
            tc.tile_pool(name="const", bufs=1) as cpool,
            tc.tile_pool(name="dram", bufs=1, space="DRAM") as dram,
        ):

            iota_sb = cpool.tile([128, 128], F32)
            nc.sync.dma_start(out=iota_sb[:], in_=iota_in[:])
            ident = cpool.tile([128, 128], F32)
            make_identity(nc, ident[:])
            w1h_sb = cpool.tile([128, KC1, D1], F32)
            nc.sync.dma_start(out=w1h_sb[:],
                              in_=W1h.rearrange("(r p) d -> p r d", p=128))
            w2h_sb = cpool.tile([128, KC2, D2], F32)
            nc.sync.dma_start(out=w2h_sb[:],
                              in_=W2h.rearrange("(r p) d -> p r d", p=128))
            il_sb = cpool.tile([128, wpc * T_LO * 8], I16)
            nc.sync.dma_start(out=il_sb[:], in_=idxlo[:])
            if T_HI:
                ih_sb = cpool.tile([128, wpc * T_HI * 8], I16)
                nc.sync.dma_start(out=ih_sb[:], in_=idxhi[:])
            id_sb = cpool.tile([128, wpc * T * 8], I16)
            nc.sync.dma_start(out=id_sb[:], in_=idxd[:])
            dl_sb = cpool.tile([128, wpc * T], F32)
            nc.sync.dma_start(out=dl_sb[:], in_=dstl[:])
            zall = cpool.tile([128, wpc], F32)
            nc.vector.memset(zall[:], 0.0)
            nbias2 = cpool.tile([128, 1], F32)
            nc.vector.memset(nbias2[:], -const2)

            # ---------------- dense layer 1 ----------------
            for _rep in range(reps):
             HDT = BF16 if BF16_H else F32
             tab_h = dram.tile([NPAD, HC], HDT, addr_space="Shared",
                               tag=f"tab_h{_rep}")
             tab_a1 = dram.tile([NPAD, 64], F32, addr_space="Shared",
                                tag=f"tab_a1{_rep}")
             tab_a2 = dram.tile([NPAD, 64], F32, addr_space="Shared",
                                tag=f"tab_a2{_rep}")
             tab_h_in = dram.tile([NPC, HC], HDT, tag=f"tab_h_in{_rep}")
             tab_a1_in = dram.tile([NPC, 64], F32, tag=f"tab_a1_in{_rep}")
             tab_a2_in = dram.tile([NPC, 64], F32, tag=f"tab_a2_in{_rep}")
             with (
                 tc.tile_pool(name="d1sb", bufs=3) as dsb,
                 tc.tile_pool(name="d1ps", bufs=2, space="PSUM") as dps,
                 tc.tile_pool(name="xtp", bufs=1) as xtp,
             ):
                 xT_sb = xtp.tile([128, KC1, NPC], F32)
                 nc.sync.dma_start(out=xT_sb[:],
                                   in_=xT.rearrange("(r p) n -> p r n", p=128))
                 for nt in range(0 if SKIP_DENSE else wpc):
                     ps = dps.tile([128, D1], F32, tag="d1")
                     for r in range(KC1):
                         nc.tensor.matmul(
                             ps[:],
                             lhsT=xT_sb[:, r, nt * 128:(nt + 1) * 128],
                             rhs=w1h_sb[:, r, :],
                             start=(r == 0), stop=(r == KC1 - 1))
                     h1t = dsb.tile([128, D1], HDT, tag="h1t")
                     nc.vector.tensor_copy(out=h1t[:, 0:HC], in_=ps[:, 0:HC])
                     aux = dsb.tile([128, 64], F32, tag="aux")
                     nc.vector.memset(aux[:, 16:64], 0.0)
                     nc.vector.tensor_copy(out=aux[:, 0:16], in_=ps[:, HC:HC + 16])
                     nc.sync.dma_start(out=tab_h_in[nt * 128:(nt + 1) * 128, :],
                                       in_=h1t[:, 0:HC])
                     nc.sync.dma_start(out=tab_a1_in[nt * 128:(nt + 1) * 128, :],
                                       in_=aux[:])

             nc.gpsimd.collective_compute(
                 "AllGather", OP.bypass, replica_groups=[list(range(N_CORES))],
                 ins=[tab_h_in.opt()], outs=[tab_h.opt()])
             nc.gpsimd.collective_compute(
                 "AllGather", OP.bypass, replica_groups=[list(range(N_CORES))],
                 ins=[tab_a1_in.opt()], outs=[tab_a1.opt()])

             # ---------------- edge stage L1 + inline dense L2 ----------------
             with (
                 tc.tile_pool(name="e1big", bufs=2) as ebig,
                 tc.tile_pool(name="e1sm", bufs=2) as esm,
                 tc.tile_pool(name="e1ps", bufs=2, space="PSUM") as eps_p,
             ):
                 for w in range(0 if SKIP_E1 else wpc):
                     gh = ebig.tile([128, T, HC], HDT, tag="gh")
                     nc.gpsimd.dma_gather(
                         gh[:, 0:T_LO, :], tab_h[:],
                         il_sb[:, w * T_LO * 8:(w + 1) * T_LO * 8],
                         T_LO * 128, T_LO * 128, HC, single_packet=False)
                     if T_HI:
                         nc.gpsimd.dma_gather(
                             gh[:, T_LO:T, :], tab_h[LOWCAP:, :],
                             ih_sb[:, w * T_HI * 8:(w + 1) * T_HI * 8],
                             T_HI * 128, T_HI * 128, HC, single_packet=False)
                     gas = esm.tile([128, T, 64], F32, tag="gas")
                     nc.gpsimd.dma_gather(
                         gas[:, 0:T_LO, :], tab_a1[:],
                         il_sb[:, w * T_LO * 8:(w + 1) * T_LO * 8],
                         T_LO * 128, T_LO * 128, 64, single_packet=False)
                     if T_HI:
                         nc.gpsimd.dma_gather(
                             gas[:, T_LO:T, :], tab_a1[LOWCAP:, :],
                             ih_sb[:, w * T_HI * 8:(w + 1) * T_HI * 8],
                             T_HI * 128, T_HI * 128, 64, single_packet=False)
                     gad = esm.tile([128, T, 64], F32, tag="gad")
                     nc.gpsimd.dma_gather(
                         gad[:], tab_a1_in[:],
                         id_sb[:, w * T * 8:(w + 1) * T * 8],
                         T * 128, T * 128, 64, single_packet=False)
                     if SKIP_E1C:
                         nc.sync.dma_start(
                             out=tab_h_in[w * 128:(w + 1) * 128, 0:16],
                             in_=gh[:, 0, 0:16])
                         nc.sync.dma_start(
                             out=tab_a2_in[w * 128:(w + 1) * 128, 16:24],
                             in_=gas[:, 0, 0:8])
                         nc.sync.dma_start(
                             out=tab_a2_in[w * 128:(w + 1) * 128, 24:32],
                             in_=gad[:, 0, 0:8])
                         continue
                     S = ebig.tile([128, T, 128], HDT, tag="S")
                     nc.vector.tensor_tensor(
                         out=S[:],
                         in0=dl_sb[:, w * T:(w + 1) * T].unsqueeze(-1)
                             .broadcast_to([128, T, 128]),
                         in1=iota_sb[:].unsqueeze(1).broadcast_to([128, T, 128]),
                         op=OP.is_equal)
                     q = esm.tile([128, T, H], F32, tag="q")
                     nc.vector.tensor_tensor(out=q[:], in0=gas[:, :, 0:H],
                                             in1=gad[:, :, H:2 * H], op=OP.add)
                     q2 = esm.tile([128, T, H], F32, tag="q2")
                     nc.vector.tensor_scalar_mul(q2[:], q[:], NEG_SLOPE)
                     e = esm.tile([128, T, H], F32, tag="e")
                     nc.vector.tensor_tensor(out=e[:], in0=q[:], in1=q2[:],
                                             op=OP.max)
                     wx = esm.tile([128, T, H], HDT, tag="wx")
                     nc.scalar.activation(wx[:], e[:], AF.Exp)
                     msg = ebig.tile([128, T, HC], HDT, tag="msg")
                     nc.vector.tensor_tensor(
                         out=msg[:].rearrange("p t (h c) -> p t h c", c=C),
                         in0=gh[:].rearrange("p t (h c) -> p t h c", c=C),
                         in1=wx[:].unsqueeze(-1).broadcast_to([128, T, H, C]),
                         op=OP.mult)

                     ps_n = eps_p.tile([128, HC], F32, tag="num")
                     ps_d = eps_p.tile([128, H], F32, tag="den")
                     for t in range(T):
                         lhs = S[:, t, :]
                         nc.tensor.matmul(ps_n[:], lhsT=lhs,
                                          rhs=msg[:, t, :],
                                          start=(t == 0), stop=(t == T - 1))
                         nc.tensor.matmul(ps_d[:], lhsT=lhs, rhs=wx[:, t, :],
                                          start=(t == 0), stop=(t == T - 1))
                     den = esm.tile([128, H], F32, tag="dn")
                     nc.vector.tensor_scalar_add(den[:], ps_d[:], EPS)
                     rden = esm.tile([128, H], F32, tag="rd")
                     nc.vector.reciprocal(rden[:], den[:])
                     h1r = esm.tile([128, HC], F32, tag="h1r")
                     nc.vector.tensor_tensor(
                         out=h1r[:].rearrange("p (h c) -> p h c", c=C),
                         in0=ps_n[:].rearrange("p (h c) -> p h c", c=C),
                         in1=rden[:].unsqueeze(-1).broadcast_to([128, H, C]),
                         op=OP.mult)
                     nc.scalar.activation(h1r[:], h1r[:], AF.Relu)

                     # inline dense layer 2 for this window's 128 nodes
                     hT = esm.tile([128, KC2, 128], F32, tag="hT")
                     for r in range(KC2):
                         pT = eps_p.tile([128, 128], F32, tag="pT")
                         nc.tensor.transpose(out=pT[:],
                                             in_=h1r[:, r * 128:(r + 1) * 128],
                                             identity=ident[:])
                         nc.scalar.copy(hT[:, r, :], pT[:])
                     ps2 = eps_p.tile([128, D2], F32, tag="d2")
                     for r in range(KC2):
                         nc.tensor.matmul(ps2[:], lhsT=hT[:, r, :],
                                          rhs=w2h_sb[:, r, :],
                                          start=(r == 0), stop=(r == KC2 - 1))
                     aux2 = esm.tile([128, 64], F32, tag="a2")
                     nc.vector.memset(aux2[:, D2:64], 0.0)
                     nc.vector.tensor_copy(out=aux2[:, 0:D2], in_=ps2[:])
                     nc.sync.dma_start(out=tab_a2_in[w * 128:(w + 1) * 128, :],
                                       in_=aux2[:])

             nc.gpsimd.collective_compute(
                 "AllGather", OP.bypass, replica_groups=[list(range(N_CORES))],
                 ins=[tab_a2_in.opt()], outs=[tab_a2.opt()])

             # ---------------- edge stage L2 ----------------
             with (
                 tc.tile_pool(name="e2big", bufs=2) as ebig2,
                 tc.tile_pool(name="e2sm", bufs=2) as esm2,
                 tc.tile_pool(name="e2ps", bufs=2, space="PSUM") as eps2,
             ):
                 for w in range(0 if SKIP_E2 else wpc):
                     gas = esm2.tile([128, T, 64], F32, tag="gas2")
                     nc.gpsimd.dma_gather(
                         gas[:, 0:T_LO, :], tab_a2[:],
                         il_sb[:, w * T_LO * 8:(w + 1) * T_LO * 8],
                         T_LO * 128, T_LO * 128, 64, single_packet=False)
                     if T_HI:
                         nc.gpsimd.dma_gather(
                             gas[:, T_LO:T, :], tab_a2[LOWCAP:, :],
                             ih_sb[:, w * T_HI * 8:(w + 1) * T_HI * 8],
                             T_HI * 128, T_HI * 128, 64, single_packet=False)
                     gad = esm2.tile([128, T, 64], F32, tag="gad2")
                     nc.gpsimd.dma_gather(
                         gad[:], tab_a2_in[:],
                         id_sb[:, w * T * 8:(w + 1) * T * 8],
                         T * 128, T * 128, 64, single_packet=False)

                     S = ebig2.tile([128, T, 128], F32, tag="S2")
                     nc.vector.tensor_tensor(
                         out=S[:],
                         in0=dl_sb[:, w * T:(w + 1) * T].unsqueeze(-1)
                             .broadcast_to([128, T, 128]),
                         in1=iota_sb[:].unsqueeze(1).broadcast_to([128, T, 128]),
                         op=OP.is_equal)
                     q = esm2.tile([128, T, H], F32, tag="q_2")
                     nc.vector.tensor_tensor(out=q[:], in0=gas[:, :, 0:H],
                                             in1=gad[:, :, H:2 * H], op=OP.add)
                     q2 = esm2.tile([128, T, H], F32, tag="q2_2")
                     nc.vector.tensor_scalar_mul(q2[:], q[:], NEG_SLOPE)
                     e = esm2.tile([128, T, H], F32, tag="e_2")
                     nc.vector.tensor_tensor(out=e[:], in0=q[:], in1=q2[:],
                                             op=OP.max)
                     wx = esm2.tile([128, T, H], F32, tag="wx2")
                     nc.scalar.activation(wx[:], e[:], AF.Exp)
                     wp = esm2.tile([128, T, H], F32, tag="wp2")
                     nc.vector.tensor_tensor(out=wp[:], in0=wx[:],
                                             in1=gas[:, :, 2 * H:3 * H],
                                             op=OP.mult)
                     ps_n = eps2.tile([128, H], F32, tag="num2")
                     ps_d = eps2.tile([128, H], F32, tag="den2")
                     for t in range(T):
                         lhs = S[:, t, :]
                         nc.tensor.matmul(ps_n[:], lhsT=lhs, rhs=wp[:, t, :],
                                          start=(t == 0), stop=(t == T - 1))
                         nc.tensor.matmul(ps_d[:], lhsT=lhs, rhs=wx[:, t, :],
                                          start=(t == 0), stop=(t == T - 1))
                     den = esm2.tile([128, H], F32, tag="dn2")
                     nc.vector.tensor_scalar_add(den[:], ps_d[:], EPS)
                     rden = esm2.tile([128, H], F32, tag="rd2")
                     nc.vector.reciprocal(rden[:], den[:])
                     o2 = esm2.tile([128, H], F32, tag="o2")
                     nc.vector.tensor_tensor(out=o2[:], in0=ps_n[:], in1=rden[:],
                                             op=OP.mult)
                     zs = esm2.tile([128, 1], F32, tag="zs")
                     nc.vector.tensor_reduce(out=zs[:], in_=o2[:],
                                             axis=mybir.AxisListType.X, op=OP.add)
                     # sigmoid(z) = 1 / (1 + exp(-z)),  z = zs/H + const2
                     ze = esm2.tile([128, 1], F32, tag="ze")
                     nc.scalar.activation(ze[:], zs[:], AF.Exp,
                                          scale=-1.0 / H, bias=nbias2[:, 0:1])
                     zp = esm2.tile([128, 1], F32, tag="zp")
                     nc.vector.tensor_scalar_add(zp[:], ze[:], 1.0)
                     nc.vector.reciprocal(zall[:, w:w + 1], zp[:])

            nc.sync.dma_start(
                out=zout.rearrange("(w p) one -> p (w one)", p=128),
                in_=zall[:])

    nc.compile()
    return nc


class _Runner:
    """Cached jitted executable for the SPMD module (mirrors
    bass2jax.run_bass_via_pjrt, but reusable across calls so repeat
    invocations skip the XLA retrace and host->device input staging)."""

    def __init__(self, nc):
        import jax
        import concourse.mybir as _mybir
        from concourse import bass2jax
        from jax.sharding import Mesh, PartitionSpec
        from jax.experimental.shard_map import shard_map

        bass2jax.install_neuronx_cc_hook()
        self.nc = nc
        pname = nc.partition_id_tensor.name if nc.partition_id_tensor else None
        in_names, out_names, out_avals, zero_outs = [], [], [], []
        for alloc in nc.m.functions[0].allocations:
            if not isinstance(alloc, _mybir.MemoryLocationSet):
                continue
            name = alloc.memorylocations[0].name
            if alloc.kind == "ExternalInput":
                if name == pname:
                    continue
                in_names.append(name)
            elif alloc.kind == "ExternalOutput":
                shape = tuple(alloc.tensor_shape)
                dtype = _mybir.dt.np(alloc.dtype)
                out_names.append(name)
                out_avals.append(jax.core.ShapedArray(shape, dtype))
                zero_outs.append(np.zeros(shape, dtype))
        n_params = len(in_names)
        all_names = in_names + out_names
        if pname is not None:
            all_names = all_names + [pname]
        donate = tuple(range(n_params, n_params + len(out_names)))

        def _body(*args):
            operands = list(args)
            if pname is not None:
                operands.append(bass2jax.partition_id_tensor())
            outs = bass2jax._bass_exec_p.bind(
                *operands,
                out_avals=tuple(out_avals),
                in_names=tuple(all_names),
                out_names=tuple(out_names),
                lowering_input_output_aliases=(),
                sim_require_finite=True,
                sim_require_nnan=True,
                nc=nc,
            )
            return tuple(outs)

        devices = jax.devices()[:N_CORES]
        self.mesh = Mesh(np.asarray(devices), ("core",))
        in_specs = (PartitionSpec("core"),) * (n_params + len(out_names))
        out_specs = (PartitionSpec("core"),) * len(out_names)
        self.fn = jax.jit(
            shard_map(_body, mesh=self.mesh, in_specs=in_specs,
                      out_specs=out_specs, check_rep=False),
            donate_argnums=donate, keep_unused=True)
        self.in_names = in_names
        self.out_names = out_names
        self.out_avals = out_avals
        self.zero_outs = zero_outs
        self.staged = None

    def stage(self, in_maps):
        """Concat per-core inputs and push them to device once."""
        import jax
        concat_in = [
            np.concatenate([np.asarray(in_maps[c][n]) for c in range(N_CORES)],
                           axis=0)
            for n in self.in_names
        ]
        self.staged = [jax.device_put(a) for a in concat_in]
        for a in self.staged:
            a.block_until_ready()

    def run(self):
        import jax
        zeros = [np.zeros((N_CORES * z.shape[0], *z.shape[1:]), z.dtype)
                 for z in self.zero_outs]
        outs = self.fn(*self.staged, *zeros)
        outs = [np.asarray(o) for o in outs]
        return {
            n: outs[i].reshape(N_CORES, *self.out_avals[i].shape)
            for i, n in enumerate(self.out_names)
        }


def _get_runner(meta, reps=1):
    key = (meta["N"], meta["F"], meta["H"], meta["C"], meta["T_LO"],
           meta["T_HI"], round(meta["const2"], 12), reps)
    if key not in _CACHE:
        nc = _build(meta, reps=reps)
        _CACHE[key] = _Runner(nc)
    return _CACHE[key]


def kernel(edge_index, x, W1, as1, ad1, b1, W2, as2, ad2, b2, Wp, bp):
    global last_exec_seconds
    import time

    meta, in_maps = _prep_host(edge_index, x, W1, as1, ad1, b1,
                               W2, as2, ad2, b2, Wp, bp)
    runner = _get_runner(meta)
    runner.stage(in_maps)
    t0 = time.time()
    res = runner.run()
    last_exec_seconds = time.time() - t0

    N = meta["N"]
    out = res["zout"].reshape(-1, 1)[:N]
    return np.ascontiguousarray(out, dtype=np.float32)


def benchmark(np_inputs, iters=5, reps=1):
    """Stage inputs once, run repeatedly; returns (seconds_list, out)."""
    import time
    meta, in_maps = _prep_host(**np_inputs)
    runner = _get_runner(meta, reps=reps)
    runner.stage(in_maps)
    runner.run()  # warm
    times = []
    res = None
    for _ in range(iters):
        t0 = time.time()
        res = runner.run()
        times.append(time.time() - t0)
    out = res["zout"].reshape(-1, 1)[:meta["N"]]
    return times, np.ascontiguousarray(out, dtype=np.float32)



# revision 3
# speedup vs baseline: 32.1931x; 32.1931x over previous
"""Trainium2 Bass kernel for a 2-layer GAT (PyG GATConv semantics) + sigmoid head.

v2 strategy (8 NeuronCores, SPMD, single NEFF launch):
  - Nodes block-sharded: core k owns `wpc` windows of 128 nodes.  Edges
    (with self-loops) are sorted by destination on the host and bucketed
    per (core, window); segment softmax and aggregation are core-local.
  - Dense layer 1 in bf16: [W1 | W1@Ad_blk] columns give h and the per-node
    dst-attention logit contributions in one matmul; b1 rides on an
    all-ones input row (softmax rows sum to 1).
  - Per-node h rows are AllGathered (bf16).  The edge stage gathers ONLY
    h[src] rows (bulk dma_gather, int16 lo/hi table split for >32K rows).
  - Source logits alpha_s[src] are recomputed per edge on the VectorEngine
    from the gathered h rows (mult by replicated a_src + segmented reduce)
    — no aux gather, no aux AllGather.
  - Destination logits ad[dst] are produced per edge on the TensorEngine:
    the selection matrix S (edge x node) is transposed tile-by-tile and
    matmul'd with the window's [128, H] dst-logit tile — no gather.
  - Segment softmax numerator/denominator via selection-matrix matmuls
    accumulated in PSUM.  exp without max-subtraction: logits are O(1),
    far from fp32 overflow, and softmax is shift-invariant.
  - Layer 2 never materializes h2: Wp folded per head into the aux table
    (as2 | ad2 | p), stored bf16 with 256-byte rows; ONE gather per edge.

kernel(**inputs) takes the FULL inputs and returns the FULL [N, 1] output.
"""

import math

import numpy as np

import concourse.bacc as bacc
import concourse.mybir as mybir
import concourse.tile as tile
from concourse.masks import make_identity

F32 = mybir.dt.float32
BF16 = mybir.dt.bfloat16
I16 = mybir.dt.int16

N_CORES = 8
WIN = 128
LOWCAP = 32768
EPS = 1e-16
NEG_SLOPE = 0.2

_CACHE: dict = {}

last_exec_seconds = None


def _pack_idx(vals: np.ndarray, nidx: int) -> np.ndarray:
    """int16 index grid for dma_gather: index j at [16g + j%16, j//16]."""
    assert vals.shape[0] == nidx and nidx % 16 == 0
    a = vals.astype(np.int16).reshape(nidx // 16, 16).T
    return np.tile(a, (8, 1))


def _blk(a: np.ndarray, H: int, C: int) -> np.ndarray:
    """[H, C] head params -> [H*C, H] block-diagonal matrix."""
    out = np.zeros((H * C, H), np.float32)
    for h in range(H):
        out[h * C:(h + 1) * C, h] = a[h]
    return out


def _to_bf16(a: np.ndarray) -> np.ndarray:
    import ml_dtypes
    return a.astype(ml_dtypes.bfloat16)


def _prep_host(edge_index, x, W1, as1, ad1, b1, W2, as2, ad2, b2, Wp, bp):
    N, F = x.shape
    H, C = as1.shape
    HC = H * C
    wpc = math.ceil(N / (N_CORES * WIN))
    NPC = wpc * WIN
    NPAD = N_CORES * NPC
    split = NPAD > LOWCAP

    # ---- augmented weights ----
    W1h = np.concatenate(
        [W1.astype(np.float32),
         W1.astype(np.float32) @ _blk(ad1, H, C)], axis=1)          # [F, HC+H]
    D1 = HC + H
    K1 = F + 1
    K1p = math.ceil(K1 / 128) * 128
    W1h_pad = np.zeros((K1p, D1), np.float32)
    W1h_pad[:F] = W1h
    W1h_pad[F, :HC] = b1

    W2f = W2.astype(np.float32)
    W2h = np.concatenate(
        [W2f @ _blk(as2, H, C), W2f @ _blk(ad2, H, C),
         W2f @ _blk(np.tile(Wp[:, 0][None, :], (H, 1)), H, C)], axis=1)  # [HC,3H]
    D2 = 3 * H
    const2 = float(np.asarray(b2, np.float64) @ np.asarray(Wp[:, 0], np.float64)
                   + np.asarray(bp, np.float64)[0])

    asrep = np.tile(as1.reshape(1, HC).astype(np.float32), (128, 1))  # [128,HC]

    # ---- edges ----
    loops = np.arange(N, dtype=np.int64)
    src = np.concatenate([np.asarray(edge_index[0]), loops])
    dst = np.concatenate([np.asarray(edge_index[1]), loops])
    order = np.argsort(dst, kind="stable")
    src_s = src[order]
    dst_s = dst[order]
    G = N_CORES * wpc
    wb = np.searchsorted(dst_s, np.arange(0, G * WIN + 1, WIN))

    lo_l, hi_l = [], []
    for g in range(G):
        s = src_s[wb[g]:wb[g + 1]]
        d = (dst_s[wb[g]:wb[g + 1]] - g * WIN).astype(np.int64)
        if split:
            m = s < LOWCAP
            lo_l.append((s[m], d[m]))
            hi_l.append((s[~m] - LOWCAP, d[~m]))
        else:
            lo_l.append((s, d))
            hi_l.append((s[:0], d[:0]))

    T_LO = max(1, max(math.ceil(len(a[0]) / WIN) for a in lo_l))
    T_HI = max(math.ceil(len(a[0]) / WIN) for a in hi_l) if split else 0
    T = T_LO + T_HI

    idxlo = np.zeros((N_CORES, 128, wpc * T_LO * 8), np.int16)
    idxhi = np.zeros((N_CORES, 128, max(1, wpc * T_HI * 8)), np.int16)
    dstl = np.full((N_CORES, 128, wpc * T), 200.0, np.float32)

    for k in range(N_CORES):
        for w in range(wpc):
            g = k * wpc + w
            (slo, dlo), (shi, dhi) = lo_l[g], hi_l[g]
            nlo, nhi = len(slo), len(shi)
            flo = np.zeros(T_LO * WIN, np.int64)
            flo[:nlo] = slo
            idxlo[k, :, w * T_LO * 8:(w + 1) * T_LO * 8] = _pack_idx(flo, T_LO * WIN)
            if T_HI:
                fhi = np.zeros(T_HI * WIN, np.int64)
                fhi[:nhi] = shi
                idxhi[k, :, w * T_HI * 8:(w + 1) * T_HI * 8] = _pack_idx(fhi, T_HI * WIN)
            # full window slot order: [lo | lo-pad | hi | hi-pad]
            dall = np.full(T * WIN, 200.0, np.float32)
            dall[:nlo] = dlo
            dall[T_LO * WIN:T_LO * WIN + nhi] = dhi
            dstl[k, :, w * T:(w + 1) * T] = dall.reshape(T, WIN).T

    # ---- per-core dense inputs (transposed, bf16) ----
    xT = np.zeros((N_CORES, K1p, NPC), np.float32)
    xnp = np.asarray(x, np.float32)
    for k in range(N_CORES):
        lo = k * NPC
        hi = min(N, (k + 1) * NPC)
        if hi > lo:
            xT[k, :F, :hi - lo] = xnp[lo:hi].T
        xT[k, F, :] = 1.0

    iota = np.tile(np.arange(128, dtype=np.float32)[None, :], (128, 1))

    meta = dict(N=N, F=F, H=H, C=C, HC=HC, wpc=wpc, NPC=NPC, NPAD=NPAD,
                split=split, T_LO=T_LO, T_HI=T_HI, T=T, K1p=K1p, D1=D1, D2=D2,
                const2=const2)
    in_maps = []
    w1h_bf = _to_bf16(W1h_pad)
    asrep_bf = _to_bf16(asrep)
    for k in range(N_CORES):
        in_maps.append({
            "xT": _to_bf16(xT[k]), "idxlo": idxlo[k], "idxhi": idxhi[k],
            "dstl": dstl[k],
            "W1h": w1h_bf, "W2h": W2h.astype(np.float32),
            "asrep": asrep_bf, "iota": iota,
        })
    return meta, in_maps


def _build(meta, reps=1):
    """Build + compile the SPMD Bass module.

    reps > 1 unrolls the whole computation body (including all input SBUF
    loads, table writes and collectives) multiple times inside one NEFF,
    so steady-state per-execution device time can be measured as a slope,
    amortizing the per-launch NRT/PJRT overhead."""
    H, C, HC = meta["H"], meta["C"], meta["HC"]
    wpc, NPC, NPAD = meta["wpc"], meta["NPC"], meta["NPAD"]
    T_LO, T_HI, T = meta["T_LO"], meta["T_HI"], meta["T"]
    K1p, D1, D2 = meta["K1p"], meta["D1"], meta["D2"]
    split, const2 = meta["split"], meta["const2"]
    KC1 = K1p // 128
    KC2 = HC // 128

    nc = bacc.Bacc("TRN2", target_bir_lowering=False, debug=False,
                   num_devices=N_CORES, num_swdge_queues=4)
    xT = nc.dram_tensor("xT", [K1p, NPC], BF16, kind="ExternalInput").ap()
    idxlo = nc.dram_tensor("idxlo", [128, wpc * T_LO * 8], I16,
                           kind="ExternalInput").ap()
    idxhi = nc.dram_tensor("idxhi", [128, max(1, wpc * T_HI * 8)], I16,
                           kind="ExternalInput").ap()
    dstl = nc.dram_tensor("dstl", [128, wpc * T], F32, kind="ExternalInput").ap()
    W1h = nc.dram_tensor("W1h", [K1p, D1], BF16, kind="ExternalInput").ap()
    W2h = nc.dram_tensor("W2h", [HC, D2], F32, kind="ExternalInput").ap()
    asrep_in = nc.dram_tensor("asrep", [128, HC], BF16, kind="ExternalInput").ap()
    iota_in = nc.dram_tensor("iota", [128, 128], F32, kind="ExternalInput").ap()
    zout = nc.dram_tensor("zout", [NPC, 1], F32, kind="ExternalOutput").ap()

    AF = mybir.ActivationFunctionType
    OP = mybir.AluOpType

    with tile.TileContext(nc) as tc:
        with (
            tc.tile_pool(name="const", bufs=1) as cpool,
            tc.tile_pool(name="dram", bufs=1, space="DRAM") as dram,
        ):
            iota_sb = cpool.tile([128, 128], F32)
            nc.sync.dma_start(out=iota_sb[:], in_=iota_in[:])
            ident = cpool.tile([128, 128], F32)
            make_identity(nc, ident[:])
            identb = cpool.tile([128, 128], BF16)
            make_identity(nc, identb[:])
            zall = cpool.tile([128, wpc], F32)
            nc.vector.memset(zall[:], 0.0)
            nbias2 = cpool.tile([128, 1], F32)
            nc.vector.memset(nbias2[:], -const2)

            for _rep in range(reps):
             tab_h = dram.tile([NPAD, HC], BF16, addr_space="Shared",
                               tag=f"tab_h{_rep}")
             tab_a2 = dram.tile([NPAD, 128], BF16, addr_space="Shared",
                                tag=f"tab_a2{_rep}")
             tab_h_in = dram.tile([NPC, HC], BF16, tag=f"tab_h_in{_rep}")
             tab_ad_in = dram.tile([NPC, 8], BF16, tag=f"tab_ad_in{_rep}")
             tab_a2_in = dram.tile([NPC, 128], BF16, tag=f"tab_a2_in{_rep}")

             with tc.tile_pool(name="cs", bufs=1) as cs:
                w1h_sb = cs.tile([128, KC1, D1], BF16)
                nc.sync.dma_start(out=w1h_sb[:],
                                  in_=W1h.rearrange("(r p) d -> p r d", p=128))
                w2h_sb = cs.tile([128, KC2, D2], F32)
                nc.sync.dma_start(out=w2h_sb[:],
                                  in_=W2h.rearrange("(r p) d -> p r d", p=128))
                asrep_sb = cs.tile([128, HC], BF16)
                nc.sync.dma_start(out=asrep_sb[:], in_=asrep_in[:])
                il_sb = cs.tile([128, wpc * T_LO * 8], I16)
                nc.sync.dma_start(out=il_sb[:], in_=idxlo[:])
                if T_HI:
                    ih_sb = cs.tile([128, wpc * T_HI * 8], I16)
                    nc.sync.dma_start(out=ih_sb[:], in_=idxhi[:])
                dl_sb = cs.tile([128, wpc * T], F32)
                nc.sync.dma_start(out=dl_sb[:], in_=dstl[:])

                # ---------------- dense layer 1 (bf16) ----------------
                with (
                    tc.tile_pool(name="d1sb", bufs=3) as dsb,
                    tc.tile_pool(name="d1ps", bufs=2, space="PSUM") as dps,
                    tc.tile_pool(name="xtp", bufs=1) as xtp,
                ):
                    xT_sb = xtp.tile([128, KC1, NPC], BF16)
                    nc.sync.dma_start(
                        out=xT_sb[:],
                        in_=xT.rearrange("(r p) n -> p r n", p=128))
                    for nt in range(wpc):
                        ps = dps.tile([128, D1], F32, tag="d1")
                        for r in range(KC1):
                            nc.tensor.matmul(
                                ps[:],
                                lhsT=xT_sb[:, r, nt * 128:(nt + 1) * 128],
                                rhs=w1h_sb[:, r, :],
                                start=(r == 0), stop=(r == KC1 - 1))
                        h1t = dsb.tile([128, HC], BF16, tag="h1t")
                        nc.vector.tensor_copy(out=h1t[:], in_=ps[:, 0:HC])
                        adt = dsb.tile([128, 8], BF16, tag="adt")
                        nc.vector.tensor_copy(out=adt[:], in_=ps[:, HC:HC + H])
                        nc.sync.dma_start(
                            out=tab_h_in[nt * 128:(nt + 1) * 128, :],
                            in_=h1t[:])
                        nc.sync.dma_start(
                            out=tab_ad_in[nt * 128:(nt + 1) * 128, :],
                            in_=adt[:])

                nc.gpsimd.collective_compute(
                    "AllGather", OP.bypass,
                    replica_groups=[list(range(N_CORES))],
                    ins=[tab_h_in.opt()], outs=[tab_h.opt()])

                # ---------- edge stage L1 + inline dense L2 ----------
                with (
                    tc.tile_pool(name="e1big", bufs=2) as ebig,
                    tc.tile_pool(name="e1sm", bufs=2) as esm,
                    tc.tile_pool(name="e1ps", bufs=2, space="PSUM") as eps_p,
                    tc.tile_pool(name="e1pt", bufs=1, space="PSUM") as eps_t,
                ):
                    for w in range(wpc):
                        gh = ebig.tile([128, T, HC], BF16, tag="gh")
                        nc.gpsimd.dma_gather(
                            gh[:, 0:T_LO, :], tab_h[:],
                            il_sb[:, w * T_LO * 8:(w + 1) * T_LO * 8],
                            T_LO * 128, T_LO * 128, HC, single_packet=False,
                            queue_num=w % 2)
                        if T_HI:
                            nc.gpsimd.dma_gather(
                                gh[:, T_LO:T, :], tab_h[LOWCAP:, :],
                                ih_sb[:, w * T_HI * 8:(w + 1) * T_HI * 8],
                                T_HI * 128, T_HI * 128, HC, single_packet=False,
                                queue_num=2 + (w % 2))
                        adw = esm.tile([128, 8], BF16, tag="adw")
                        nc.sync.dma_start(
                            out=adw[:],
                            in_=tab_ad_in[w * 128:(w + 1) * 128, :])

                        # selection matrix S[e, n] and its transpose S2[n, e]
                        S = ebig.tile([128, T, 128], BF16, tag="S")
                        nc.vector.tensor_tensor(
                            out=S[:],
                            in0=dl_sb[:, w * T:(w + 1) * T].unsqueeze(-1)
                                .broadcast_to([128, T, 128]),
                            in1=iota_sb[:].unsqueeze(1)
                                .broadcast_to([128, T, 128]),
                            op=OP.is_equal)
                        S2 = ebig.tile([128, T, 128], BF16, tag="S2")
                        ad_ps = eps_t.tile([128, T * 8], F32, tag="adps")
                        for t in range(T):
                            pT = eps_t.tile([128, 128], BF16, tag="pT")
                            nc.tensor.transpose(out=pT[:], in_=S[:, t, :],
                                                identity=identb[:])
                            nc.scalar.copy(S2[:, t, :], pT[:])
                            nc.tensor.matmul(
                                ad_ps[:, t * 8:(t + 1) * 8],
                                lhsT=S2[:, t, :], rhs=adw[:],
                                start=True, stop=True)
                        ad_e = esm.tile([128, T, 8], F32, tag="ade")
                        nc.vector.tensor_copy(
                            out=ad_e[:].rearrange("p t h -> p (t h)"),
                            in_=ad_ps[:])

                        # alpha_s per edge from gathered h rows
                        tmp = ebig.tile([128, T, HC], BF16, tag="tmp")
                        nc.vector.tensor_tensor(
                            out=tmp[:],
                            in0=gh[:],
                            in1=asrep_sb[:].unsqueeze(1)
                                .broadcast_to([128, T, HC]),
                            op=OP.mult)
                        as_e = esm.tile([128, T, H], F32, tag="ase")
                        nc.vector.tensor_reduce(
                            out=as_e[:].rearrange("p t h -> p (t h)")
                                .unsqueeze(-1),
                            in_=tmp[:].rearrange("p t (h c) -> p (t h) c", c=C),
                            axis=mybir.AxisListType.X, op=OP.add)

                        q = esm.tile([128, T, H], F32, tag="q")
                        nc.vector.tensor_tensor(out=q[:], in0=as_e[:],
                                                in1=ad_e[:], op=OP.add)
                        q2 = esm.tile([128, T, H], F32, tag="q2")
                        nc.vector.tensor_scalar_mul(q2[:], q[:], NEG_SLOPE)
                        e = esm.tile([128, T, H], F32, tag="e")
                        nc.vector.tensor_tensor(out=e[:], in0=q[:], in1=q2[:],
                                                op=OP.max)
                        wx = esm.tile([128, T, H], BF16, tag="wx")
                        nc.scalar.activation(wx[:], e[:], AF.Exp)
                        msg = ebig.tile([128, T, HC], BF16, tag="msg")
                        nc.vector.tensor_tensor(
                            out=msg[:].rearrange("p t (h c) -> p t h c", c=C),
                            in0=gh[:].rearrange("p t (h c) -> p t h c", c=C),
                            in1=wx[:].unsqueeze(-1).broadcast_to([128, T, H, C]),
                            op=OP.mult)

                        ps_n = eps_p.tile([128, HC], F32, tag="num")
                        ps_d = eps_p.tile([128, H], F32, tag="den")
                        for t in range(T):
                            lhs = S[:, t, :]
                            nc.tensor.matmul(ps_n[:], lhsT=lhs,
                                             rhs=msg[:, t, :],
                                             start=(t == 0), stop=(t == T - 1))
                            nc.tensor.matmul(ps_d[:], lhsT=lhs,
                                             rhs=wx[:, t, :],
                                             start=(t == 0), stop=(t == T - 1))
                        den = esm.tile([128, H], F32, tag="dn")
                        nc.vector.tensor_scalar_add(den[:], ps_d[:], EPS)
                        rden = esm.tile([128, H], F32, tag="rd")
                        nc.vector.reciprocal(rden[:], den[:])
                        h1r = esm.tile([128, HC], F32, tag="h1r")
                        nc.vector.tensor_tensor(
                            out=h1r[:].rearrange("p (h c) -> p h c", c=C),
                            in0=ps_n[:].rearrange("p (h c) -> p h c", c=C),
                            in1=rden[:].unsqueeze(-1).broadcast_to([128, H, C]),
                            op=OP.mult)
                        nc.scalar.activation(h1r[:], h1r[:], AF.Relu)

                        # inline dense layer 2 for this window's 128 nodes
                        hT = esm.tile([128, KC2, 128], F32, tag="hT")
                        for r in range(KC2):
                            pT2 = eps_t.tile([128, 128], F32, tag="pT2")
                            nc.tensor.transpose(
                                out=pT2[:], in_=h1r[:, r * 128:(r + 1) * 128],
                                identity=ident[:])
                            nc.scalar.copy(hT[:, r, :], pT2[:])
                        ps2 = eps_t.tile([128, D2], F32, tag="d2")
                        for r in range(KC2):
                            nc.tensor.matmul(ps2[:], lhsT=hT[:, r, :],
                                             rhs=w2h_sb[:, r, :],
                                             start=(r == 0), stop=(r == KC2 - 1))
                        aux2 = esm.tile([128, 128], BF16, tag="a2")
                        nc.vector.memset(aux2[:, D2:128], 0.0)
                        nc.vector.tensor_copy(out=aux2[:, 0:D2], in_=ps2[:])
                        nc.sync.dma_start(
                            out=tab_a2_in[w * 128:(w + 1) * 128, :],
                            in_=aux2[:])

                nc.gpsimd.collective_compute(
                    "AllGather", OP.bypass,
                    replica_groups=[list(range(N_CORES))],
                    ins=[tab_a2_in.opt()], outs=[tab_a2.opt()])

                # ---------------- edge stage L2 ----------------
                with (
                    tc.tile_pool(name="e2big", bufs=2) as ebig2,
                    tc.tile_pool(name="e2sm", bufs=2) as esm2,
                    tc.tile_pool(name="e2ps", bufs=2, space="PSUM") as eps2,
                ):
                    for w in range(wpc):
                        gas = ebig2.tile([128, T, 128], BF16, tag="gas2")
                        nc.gpsimd.dma_gather(
                            gas[:, 0:T_LO, :], tab_a2[:],
                            il_sb[:, w * T_LO * 8:(w + 1) * T_LO * 8],
                            T_LO * 128, T_LO * 128, 128, single_packet=False,
                            queue_num=w % 2)
                        if T_HI:
                            nc.gpsimd.dma_gather(
                                gas[:, T_LO:T, :], tab_a2[LOWCAP:, :],
                                ih_sb[:, w * T_HI * 8:(w + 1) * T_HI * 8],
                                T_HI * 128, T_HI * 128, 128,
                                single_packet=False, queue_num=2 + (w % 2))
                        adw2 = esm2.tile([128, 8], BF16, tag="adw2")
                        nc.sync.dma_start(
                            out=adw2[:],
                            in_=tab_a2_in[w * 128:(w + 1) * 128, H:2 * H])

                        S = ebig2.tile([128, T, 128], BF16, tag="S_2")
                        nc.vector.tensor_tensor(
                            out=S[:],
                            in0=dl_sb[:, w * T:(w + 1) * T].unsqueeze(-1)
                                .broadcast_to([128, T, 128]),
                            in1=iota_sb[:].unsqueeze(1)
                                .broadcast_to([128, T, 128]),
                            op=OP.is_equal)
                        S2 = ebig2.tile([128, T, 128], BF16, tag="S2_2")
                        ad_ps = eps2.tile([128, T * 8], F32, tag="adps2")
                        for t in range(T):
                            pT = eps2.tile([128, 128], BF16, tag="pT_2")
                            nc.tensor.transpose(out=pT[:], in_=S[:, t, :],
                                                identity=identb[:])
                            nc.scalar.copy(S2[:, t, :], pT[:])
                            nc.tensor.matmul(
                                ad_ps[:, t * 8:(t + 1) * 8],
                                lhsT=S2[:, t, :], rhs=adw2[:],
                                start=True, stop=True)
                        ad_e = esm2.tile([128, T, 8], F32, tag="ade2")
                        nc.vector.tensor_copy(
                            out=ad_e[:].rearrange("p t h -> p (t h)"),
                            in_=ad_ps[:])

                        as_e = esm2.tile([128, T, H], F32, tag="ase2")
                        nc.vector.tensor_copy(out=as_e[:], in_=gas[:, :, 0:H])
                        p_e = esm2.tile([128, T, H], BF16, tag="pe2")
                        nc.vector.tensor_copy(out=p_e[:],
                                              in_=gas[:, :, 2 * H:3 * H])

                        q = esm2.tile([128, T, H], F32, tag="q_2")
                        nc.vector.tensor_tensor(out=q[:], in0=as_e[:],
                                                in1=ad_e[:], op=OP.add)
                        q2 = esm2.tile([128, T, H], F32, tag="q2_2")
                        nc.vector.tensor_scalar_mul(q2[:], q[:], NEG_SLOPE)
                        e = esm2.tile([128, T, H], F32, tag="e_2")
                        nc.vector.tensor_tensor(out=e[:], in0=q[:], in1=q2[:],
                                                op=OP.max)
                        wx = esm2.tile([128, T, H], BF16, tag="wx2")
                        nc.scalar.activation(wx[:], e[:], AF.Exp)
                        wp = esm2.tile([128, T, H], BF16, tag="wp2")
                        nc.vector.tensor_tensor(out=wp[:], in0=wx[:],
                                                in1=p_e[:], op=OP.mult)

                        ps_n = eps2.tile([128, H], F32, tag="num2")
                        ps_d = eps2.tile([128, H], F32, tag="den2")
                        for t in range(T):
                            lhs = S[:, t, :]
                            nc.tensor.matmul(ps_n[:], lhsT=lhs, rhs=wp[:, t, :],
                                             start=(t == 0), stop=(t == T - 1))
                            nc.tensor.matmul(ps_d[:], lhsT=lhs, rhs=wx[:, t, :],
                                             start=(t == 0), stop=(t == T - 1))
                        den = esm2.tile([128, H], F32, tag="dn2")
                        nc.vector.tensor_scalar_add(den[:], ps_d[:], EPS)
                        rden = esm2.tile([128, H], F32, tag="rd2")
                        nc.vector.reciprocal(rden[:], den[:])
                        o2 = esm2.tile([128, H], F32, tag="o2")
                        nc.vector.tensor_tensor(out=o2[:], in0=ps_n[:],
                                                in1=rden[:], op=OP.mult)
                        zs = esm2.tile([128, 1], F32, tag="zs")
                        nc.vector.tensor_reduce(out=zs[:], in_=o2[:],
                                                axis=mybir.AxisListType.X,
                                                op=OP.add)
                        # sigmoid(z) = 1/(1+exp(-z)), z = zs/H + const2
                        ze = esm2.tile([128, 1], F32, tag="ze")
                        nc.scalar.activation(ze[:], zs[:], AF.Exp,
                                             scale=-1.0 / H, bias=nbias2[:, 0:1])
                        zp = esm2.tile([128, 1], F32, tag="zp")
                        nc.vector.tensor_scalar_add(zp[:], ze[:], 1.0)
                        nc.vector.reciprocal(zall[:, w:w + 1], zp[:])

            nc.sync.dma_start(
                out=zout.rearrange("(w p) one -> p (w one)", p=128),
                in_=zall[:])

    nc.compile()
    return nc


class _Runner:
    """Fast-dispatch pipelined executor for the SPMD module.

    Inputs and zero output-ballast buffers are staged on device once and
    reused (no donation); executions are dispatched asynchronously so M
    back-to-back runs amortize the per-launch RPC/NRT overhead."""

    def __init__(self, nc):
        import jax
        import concourse.mybir as _mybir
        from concourse import bass2jax
        from jax.sharding import Mesh, PartitionSpec
        from jax.experimental.shard_map import shard_map

        bass2jax.install_neuronx_cc_hook()
        self.nc = nc
        pname = nc.partition_id_tensor.name if nc.partition_id_tensor else None
        in_names, out_names, out_avals = [], [], []
        for alloc in nc.m.functions[0].allocations:
            if not isinstance(alloc, _mybir.MemoryLocationSet):
                continue
            name = alloc.memorylocations[0].name
            if alloc.kind == "ExternalInput":
                if name != pname:
                    in_names.append(name)
            elif alloc.kind == "ExternalOutput":
                out_names.append(name)
                out_avals.append(jax.core.ShapedArray(
                    tuple(alloc.tensor_shape), _mybir.dt.np(alloc.dtype)))
        all_names = in_names + out_names + ([pname] if pname else [])
        n_in = len(in_names)

        def _body(*args):
            operands = list(args)
            if pname is not None:
                operands.append(bass2jax.partition_id_tensor())
            outs = bass2jax._bass_exec_p.bind(
                *operands, out_avals=tuple(out_avals),
                in_names=tuple(all_names), out_names=tuple(out_names),
                lowering_input_output_aliases=(),
                sim_require_finite=True, sim_require_nnan=True, nc=nc)
            return tuple(outs)

        devices = jax.devices()[:N_CORES]
        self.mesh = Mesh(np.asarray(devices), ("core",))
        from jax.sharding import NamedSharding
        self.sh = NamedSharding(self.mesh, PartitionSpec("core"))
        self.zeros_dev = [jax.device_put(
            np.zeros((N_CORES * a.shape[0], *a.shape[1:]), a.dtype), self.sh)
            for a in out_avals]
        arg_structs = None  # filled at stage()

        self._shard_map = shard_map
        self._PartitionSpec = PartitionSpec
        self._bass2jax = bass2jax
        self._body_fn = _body
        self.in_names = in_names
        self.out_names = out_names
        self.out_avals = out_avals
        self.n_in = n_in
        self.fn = None
        self.staged = None

    def stage(self, in_maps):
        import jax
        concat_in = [
            np.concatenate([np.asarray(in_maps[c][n]) for c in range(N_CORES)],
                           axis=0)
            for n in self.in_names
        ]
        self.staged = [jax.device_put(a, self.sh) for a in concat_in]
        for a in self.staged:
            a.block_until_ready()
        if self.fn is None:
            PartitionSpec = self._PartitionSpec
            arg_structs = [jax.ShapeDtypeStruct(a.shape, a.dtype,
                                                sharding=self.sh)
                           for a in self.staged + self.zeros_dev]
            nspec = self.n_in + len(self.out_names)

            def compile_fn():
                jitted = jax.jit(
                    self._shard_map(
                        self._body_fn, mesh=self.mesh,
                        in_specs=(PartitionSpec("core"),) * nspec,
                        out_specs=(PartitionSpec("core"),) * len(self.out_names)),
                    keep_unused=True)
                return jitted.lower(*arg_structs).compile()

            try:
                self.fn = self._bass2jax.fast_dispatch_compile(compile_fn)
            except Exception:
                self.fn = jax.jit(
                    self._shard_map(
                        self._body_fn, mesh=self.mesh,
                        in_specs=(PartitionSpec("core"),) * nspec,
                        out_specs=(PartitionSpec("core"),) * len(self.out_names)),
                    keep_unused=True)

    def run_async(self):
        return self.fn(*self.staged, *self.zeros_dev)

    def run(self):
        outs = self.run_async()
        outs = [np.asarray(o) for o in outs]
        return {
            n: outs[i].reshape(N_CORES, *self.out_avals[i].shape)
            for i, n in enumerate(self.out_names)
        }


def _get_runner(meta, reps=1):
    key = (meta["N"], meta["F"], meta["H"], meta["C"], meta["T_LO"],
           meta["T_HI"], round(meta["const2"], 12), reps)
    if key not in _CACHE:
        nc = _build(meta, reps=reps)
        _CACHE[key] = _Runner(nc)
    return _CACHE[key]


def kernel(edge_index, x, W1, as1, ad1, b1, W2, as2, ad2, b2, Wp, bp):
    global last_exec_seconds
    import time

    meta, in_maps = _prep_host(edge_index, x, W1, as1, ad1, b1,
                               W2, as2, ad2, b2, Wp, bp)
    runner = _get_runner(meta)
    runner.stage(in_maps)
    t0 = time.time()
    res = runner.run()
    last_exec_seconds = time.time() - t0

    N = meta["N"]
    out = res["zout"].reshape(-1, 1)[:N]
    return np.ascontiguousarray(out, dtype=np.float32)


def benchmark(np_inputs, iters=5, reps=1, pipeline=1):
    """Stage inputs once, run repeatedly; returns (seconds_list, out).

    Each returned time is wall-clock for `pipeline` asynchronously-dispatched
    full executions divided by (pipeline * reps) — i.e. sustained seconds per
    complete evaluation of the problem on the hardware."""
    import time
    meta, in_maps = _prep_host(**np_inputs)
    runner = _get_runner(meta, reps=reps)
    runner.stage(in_maps)
    runner.run()  # warm
    times = []
    out_np = None
    for _ in range(iters):
        t0 = time.time()
        outs = [runner.run_async() for _ in range(pipeline)]
        out_np = np.asarray(outs[-1][0])
        times.append((time.time() - t0) / (pipeline * reps))
        del outs
    out = out_np.reshape(N_CORES, *runner.out_avals[0].shape)
    out = out.reshape(-1, 1)[:meta["N"]]
    return times, np.ascontiguousarray(out, dtype=np.float32)
